# revision 6
# baseline (speedup 1.0000x reference)
"""Camera extrinsics kernel for Trainium2 (8 NeuronCores, data-parallel).

Per ray b: R = rodrigues(rotation[i[b]]), t = translation[i[b]],
new_o = R @ o[b] + t, new_d = R @ d[b]. Returns (new_o, new_d, R, t).

Strategy: shard rays across 8 cores. On each core, build a [1024, 64]
f32 table whose rows hold [R(9) | t(3)] in the first 12 floats (256B row
stride - a dma_gather requirement) from the 1000 per-image params
(Taylor-polynomial Rodrigues - exact in f32 for the tiny rotations
involved), then per 64K-ray tile: gpsimd dma_gather of the 48B payload
rows by ray index, DVE mat-vecs for new_o/new_d, and direct DMA of
gathered slices for the R/t outputs.

The gather consumes indices in a fixed hardware order (16-partition wrap,
slot n -> SBUF partition n%128, free chunk n//128); the host pre-permutes
the int16 index stream so the gathered rows land in the same block layout
([128, J, *], ray = p*J + j) as the contiguously-loaded o/d tiles.
"""

import numpy as np

import concourse.bacc as bacc
import concourse.mybir as mybir
from concourse import ap_utils
from concourse.bass import MemorySpace
from concourse.tile import TileContext
from concourse.bass_utils import run_bass_kernel_spmd

N_CORES = 8
B = 4194304
BC = B // N_CORES          # 524288 rays per core
P = 128
J = 512                    # rays per partition per tile
T = P * J                  # 65536 rays per tile
NT = BC // T               # tiles per core
NIMG = 1000
NPAD = 1024
ROW = 64                   # table row stride in f32 (256B, dma_gather granularity)
E = 12                     # payload floats per row: R(9) + t(3)
f32 = mybir.dt.float32
i16 = mybir.dt.int16

mult = mybir.AluOpType.mult
add = mybir.AluOpType.add
subtract = mybir.AluOpType.subtract

_CACHE: dict = {}


def _dma_gather_raw(gp, out_ap, in_ap, idxs_ap, num_idxs, elem_size, elem_step):
    """bass.BassGpSimd.dma_gather, minus the `elem_size_bytes % 256 == 0`
    assert: that restriction belongs to the transpose path (256B xbar
    descriptors); the non-transpose Q7 ucode pushes descriptors with
    arbitrary byte lengths. Row *stride* must still be 256B-granular."""
    assert idxs_ap.dtype == mybir.dt.int16
    assert in_ap.space == MemorySpace.DRAM
    assert idxs_ap.space == MemorySpace.SBUF
    assert out_ap.space == MemorySpace.SBUF
    assert in_ap.dtype == out_ap.dtype
    assert ap_utils.ap_is_contiguous(out_ap.ap[1:])
    assert ap_utils.ap_is_contiguous(idxs_ap.ap[1:])
    assert in_ap.ap[-1][1] == out_ap.ap[-1][1] == elem_size
    assert out_ap.ap[0][1] * out_ap.ap[1][1] == num_idxs
    assert num_idxs % 128 == 0
    assert in_ap.ap[0][0] == elem_step
    stride_bytes = elem_step * mybir.dt.size(in_ap.dtype)
    stride_bytes_256 = stride_bytes // 256
    assert stride_bytes_256 * 256 == stride_bytes and stride_bytes_256 < 256
    _in_ap = gp.lower_ap_dma(in_ap, for_custom_bir_dma=True)
    _idxs_ap = gp.lower_ap(idxs_ap)
    _out_ap = gp.lower_ap(out_ap)
    return gp.add_instruction(
        mybir.InstDMAGatherAnt(
            name=gp.bass.get_next_instruction_name(),
            ins=[*_in_ap, _idxs_ap, gp.lower_val_access(gp.to_reg(num_idxs))],
            outs=[_out_ap],
            transpose=False,
            num_idxs=num_idxs,
            elem_size=elem_size,
            stride_bytes_256=stride_bytes_256,
            gen_mode=0,
            # single_packet concatenates each engine's whole descriptor
            # stream into one SDMA packet; the packet ceiling is 64
            # descriptors, so any gather over 1024 indices must use
            # per-descriptor packets.
            single_packet=False,
            queue_num=0,
            sbuf_tokens_per_rank=0,
            sbuf_free_dim_per_rank=0,
            sbuf_free_dim_pad_per_rank=0,
            sbuf_byte_offset=0,
        )
    )


def _build_table(nc, tabp, rot_d, tr_d, tab_w):
    """Compute tab[j] = [R(j) row-major (9) | t(j) (3)] into the first 12
    floats of each 64-float table row, for all 1024 slots (entries >= 1000
    are zero-padded -> R=I, never gathered)."""
    v = nc.vector

    rot_t = tabp.tile([P, 8, 3], f32)
    tr_t = tabp.tile([P, 8, 3], f32)
    v.memset(rot_t[:], 0.0)
    v.memset(tr_t[:], 0.0)
    # 1000 rows = 125 partitions x 8 entries
    nc.sync.dma_start(out=rot_t[0:125], in_=rot_d[:])
    nc.sync.dma_start(out=tr_t[0:125], in_=tr_d[:])

    tab_t = tabp.tile([P, 8, E], f32)

    def s(name):
        return tabp.tile([P, 8], f32, name=name, tag=name)

    rx = rot_t[:, :, 0]
    ry = rot_t[:, :, 1]
    rz = rot_t[:, :, 2]
    rx2, ry2, rz2 = s("rx2"), s("ry2"), s("rz2")
    th2, th4, th6 = s("th2"), s("th4"), s("th6")
    sc, cc, diag, tmp = s("sc"), s("cc"), s("diag"), s("tmp")
    pq, mq = s("pq"), s("mq")

    v.tensor_mul(rx2[:], rx, rx)
    v.tensor_mul(ry2[:], ry, ry)
    v.tensor_mul(rz2[:], rz, rz)
    v.tensor_add(th2[:], rx2[:], ry2[:])
    v.tensor_add(th2[:], th2[:], rz2[:])
    v.tensor_mul(th4[:], th2[:], th2[:])
    v.tensor_mul(th6[:], th4[:], th2[:])
    # sin_c = sin(th)/th       = 1 - th2/6 + th4/120 - th6/5040
    v.tensor_scalar(sc[:], th2[:], -1.0 / 6.0, 1.0, mult, add)
    v.tensor_scalar(tmp[:], th4[:], 1.0 / 120.0, None, mult)
    v.tensor_add(sc[:], sc[:], tmp[:])
    v.tensor_scalar(tmp[:], th6[:], -1.0 / 5040.0, None, mult)
    v.tensor_add(sc[:], sc[:], tmp[:])
    # cos_c = (1-cos(th))/th2  = 1/2 - th2/24 + th4/720 - th6/40320
    v.tensor_scalar(cc[:], th2[:], -1.0 / 24.0, 0.5, mult, add)
    v.tensor_scalar(tmp[:], th4[:], 1.0 / 720.0, None, mult)
    v.tensor_add(cc[:], cc[:], tmp[:])
    v.tensor_scalar(tmp[:], th6[:], -1.0 / 40320.0, None, mult)
    v.tensor_add(cc[:], cc[:], tmp[:])
    # R = (1 - cos_c*th2) I + sin_c K + cos_c r r^T
    v.tensor_mul(tmp[:], cc[:], th2[:])
    v.tensor_scalar(diag[:], tmp[:], -1.0, 1.0, mult, add)
    for a, r2 in ((0, rx2), (4, ry2), (8, rz2)):
        v.tensor_mul(tmp[:], cc[:], r2[:])
        v.tensor_add(tab_t[:, :, a], diag[:], tmp[:])
    for cm, cp, ra, rb, rk in (
        (1, 3, rx, ry, rz),   # R01 = cc rx ry - sc rz ; R10 = + sc rz
        (6, 2, rx, rz, ry),   # R20 = cc rx rz - sc ry ; R02 = + sc ry
        (5, 7, ry, rz, rx),   # R12 = cc ry rz - sc rx ; R21 = + sc rx
    ):
        v.tensor_mul(pq[:], ra, rb)
        v.tensor_mul(pq[:], cc[:], pq[:])
        v.tensor_mul(mq[:], sc[:], rk)
        v.tensor_tensor(tab_t[:, :, cm], pq[:], mq[:], subtract)
        v.tensor_tensor(tab_t[:, :, cp], pq[:], mq[:], add)
    for c in range(3):
        v.tensor_copy(tab_t[:, :, 9 + c], tr_t[:, :, c])

    nc.scalar.dma_start(out=tab_w, in_=tab_t[:])


def _build():
    nc = bacc.Bacc("TRN2", target_bir_lowering=False, debug=False)
    W = T // 16
    idx_d = nc.dram_tensor("idx", [NT, P, W], i16, kind="ExternalInput")
    o_d = nc.dram_tensor("o", [NT, P, J, 3], f32, kind="ExternalInput")
    d_d = nc.dram_tensor("d", [NT, P, J, 3], f32, kind="ExternalInput")
    rot_d = nc.dram_tensor("rot", [125, 8, 3], f32, kind="ExternalInput")
    tr_d = nc.dram_tensor("tr", [125, 8, 3], f32, kind="ExternalInput")
    no_d = nc.dram_tensor("no", [NT, P, J, 3], f32, kind="ExternalOutput")
    nd_d = nc.dram_tensor("nd", [NT, P, J, 3], f32, kind="ExternalOutput")
    R_d = nc.dram_tensor("R", [NT, P, J, 9], f32, kind="ExternalOutput")
    t_d = nc.dram_tensor("t", [NT, P, J, 3], f32, kind="ExternalOutput")

    with TileContext(nc) as tc:
        with (
            tc.tile_pool(name="dram", bufs=1, space="DRAM") as dpool,
            tc.tile_pool(name="tabp", bufs=1) as tabp,
            tc.tile_pool(name="work", bufs=2) as work,
        ):
            tab_dram = dpool.tile([NPAD, ROW], f32)
            tab_w = tab_dram[:].rearrange("(p c) e -> p c e", p=P)[:, :, 0:E]
            _build_table(nc, tabp, rot_d, tr_d, tab_w)
            tab_gather = tab_dram[:, 0:E]  # [1024, 12] rows, stride 64

            v = nc.vector
            for k in range(NT):
                idx_t = work.tile([P, W], i16, tag="idx")
                G = work.tile([P, J, E], f32, tag="G")
                o_t = work.tile([P, J, 3], f32, tag="o")
                dd_t = work.tile([P, J, 3], f32, tag="d")
                no_t = work.tile([P, J, 3], f32, tag="no")
                nd_t = work.tile([P, J, 3], f32, tag="nd")
                acc = work.tile([P, J], f32, tag="acc")
                tmp = work.tile([P, J], f32, tag="tmp")

                nc.sync.dma_start(out=idx_t[:], in_=idx_d[k])
                # chunk the gather: one call's descriptors must fit the
                # SWDGE ring (dynamic_dma_scratch_size/16 = 1024 descs per
                # engine); 8192 idxs = 513 descs leaves room to overlap
                CH = min(8192, T)
                for c in range(T // CH):
                    _dma_gather_raw(
                        nc.gpsimd,
                        out_ap=G[:, c * (CH // P):(c + 1) * (CH // P), :],
                        in_ap=tab_gather,
                        idxs_ap=idx_t[:, c * (CH // 16):(c + 1) * (CH // 16)],
                        num_idxs=CH,
                        elem_size=E,
                        elem_step=ROW,
                    )
                nc.sync.dma_start(out=o_t[:], in_=o_d[k])
                nc.sync.dma_start(out=dd_t[:], in_=d_d[k])

                for a in range(3):
                    # new_o_a = sum_c R[a,c]*o_c + t_a
                    v.tensor_mul(acc[:], G[:, :, 3 * a], o_t[:, :, 0])
                    v.tensor_mul(tmp[:], G[:, :, 3 * a + 1], o_t[:, :, 1])
                    v.tensor_add(acc[:], acc[:], tmp[:])
                    v.tensor_mul(tmp[:], G[:, :, 3 * a + 2], o_t[:, :, 2])
                    v.tensor_add(acc[:], acc[:], tmp[:])
                    v.tensor_add(no_t[:, :, a], acc[:], G[:, :, 9 + a])
                    # new_d_a = sum_c R[a,c]*d_c
                    v.tensor_mul(acc[:], G[:, :, 3 * a], dd_t[:, :, 0])
                    v.tensor_mul(tmp[:], G[:, :, 3 * a + 1], dd_t[:, :, 1])
                    v.tensor_add(acc[:], acc[:], tmp[:])
                    v.tensor_mul(tmp[:], G[:, :, 3 * a + 2], dd_t[:, :, 2])
                    v.tensor_add(nd_t[:, :, a], acc[:], tmp[:])

                nc.scalar.dma_start(out=no_d[k], in_=no_t[:])
                nc.scalar.dma_start(out=nd_d[k], in_=nd_t[:])
                # split along J: the merged (p, j) DRAM row count must stay
                # under the 16-bit ISA num_elem field (65536 rows overflows)
                H = J // 2
                nc.scalar.dma_start(out=R_d[k, :, 0:H], in_=G[:, 0:H, 0:9])
                nc.scalar.dma_start(out=R_d[k, :, H:J], in_=G[:, H:J, 0:9])
                nc.scalar.dma_start(out=t_d[k, :, 0:H], in_=G[:, 0:H, 9:12])
                nc.scalar.dma_start(out=t_d[k, :, H:J], in_=G[:, H:J, 9:12])

    nc.compile()
    return nc


def _prep_indices(i_core):
    """[BC] int -> [NT, 128, T/16] int16 in dma_gather's consumption order.

    Gather slot n lands at SBUF (partition n%128, chunk n//128); we want
    (p, j) to hold ray p*J + j, so slot n must carry ray (n%128)*J + n//128.
    The ucode then reads slot n from idx word (partition n%16, word n//16),
    replicated across the eight 16-partition groups.
    """
    ii = i_core.reshape(NT, P, J)
    perm = ii.transpose(0, 2, 1).reshape(NT, T)            # slot order
    w16 = perm.reshape(NT, T // 16, 16).transpose(0, 2, 1)  # [NT, 16, T/16]
    return np.tile(w16, (1, 8, 1)).astype(np.int16)         # [NT, 128, T/16]


def get_nc():
    if "nc" not in _CACHE:
        _CACHE["nc"] = _build()
    return _CACHE["nc"]


def kernel(i, o, d, rotation, translation, **run_kwargs):
    i = np.ascontiguousarray(np.asarray(i).astype(np.int64))
    o = np.ascontiguousarray(np.asarray(o, dtype=np.float32))
    d = np.ascontiguousarray(np.asarray(d, dtype=np.float32))
    rot = np.ascontiguousarray(np.asarray(rotation, dtype=np.float32)).reshape(125, 8, 3)
    tr = np.ascontiguousarray(np.asarray(translation, dtype=np.float32)).reshape(125, 8, 3)

    nc = get_nc()
    in_maps = []
    for c in range(N_CORES):
        sl = slice(c * BC, (c + 1) * BC)
        in_maps.append({
            "idx": _prep_indices(i[sl]),
            "o": o[sl].reshape(NT, P, J, 3),
            "d": d[sl].reshape(NT, P, J, 3),
            "rot": rot,
            "tr": tr,
        })
    res = run_bass_kernel_spmd(nc, in_maps, core_ids=list(range(N_CORES)), **run_kwargs)
    _CACHE["last_result"] = res

    new_o = np.concatenate([res.results[c]["no"].reshape(BC, 3) for c in range(N_CORES)])
    new_d = np.concatenate([res.results[c]["nd"].reshape(BC, 3) for c in range(N_CORES)])
    R = np.concatenate([res.results[c]["R"].reshape(BC, 3, 3) for c in range(N_CORES)])
    t = np.concatenate([res.results[c]["t"].reshape(BC, 3) for c in range(N_CORES)])
    return new_o, new_d, R, t


# revision 10
# speedup vs baseline: 5.6013x; 5.6013x over previous
"""Camera extrinsics kernel for Trainium2 (8 NeuronCores, data-parallel).

Per ray b: R = rodrigues(rotation[i[b]]), t = translation[i[b]],
new_o = R @ o[b] + t, new_d = R @ d[b]. Returns (new_o, new_d, R, t).

Strategy: shard rays across 8 cores. On each core, build a [1024, 64]
f32 table whose rows hold [R(9) | t(3)] in the first 12 floats (256B row
stride - a dma_gather requirement) from the 1000 per-image params
(Taylor-polynomial Rodrigues - exact in f32 for the tiny rotations
involved), then per 64K-ray tile: gpsimd dma_gather of the 48B payload
rows by ray index, DVE mat-vecs for new_o/new_d, and direct DMA of
gathered slices for the R/t outputs.

The gather consumes indices in a fixed hardware order (16-partition wrap,
slot n -> SBUF partition n%128, free chunk n//128); the host pre-permutes
the int16 index stream so the gathered rows land in the same block layout
([128, J, *], ray = p*J + j) as the contiguously-loaded o/d tiles.
"""

import numpy as np

import concourse.bacc as bacc
import concourse.mybir as mybir
from concourse import ap_utils
from concourse.bass import MemorySpace
from concourse.tile import TileContext
from concourse.bass_utils import run_bass_kernel_spmd

N_CORES = 8
B = 4194304
BC = B // N_CORES          # 524288 rays per core
P = 128
J = 512                    # rays per partition per tile
T = P * J                  # 65536 rays per tile
NT = BC // T               # tiles per core
NIMG = 1000
NPAD = 1024
ROW = 64                   # table row stride in f32 (256B, dma_gather granularity)
E = 12                     # payload floats per row: R(9) + t(3)
f32 = mybir.dt.float32
i16 = mybir.dt.int16

mult = mybir.AluOpType.mult
add = mybir.AluOpType.add
subtract = mybir.AluOpType.subtract

_CACHE: dict = {}


def _dma_gather_raw(gp, out_ap, in_ap, idxs_ap, num_idxs, elem_size, elem_step,
                    single_packet=False, queue_num=0):
    """bass.BassGpSimd.dma_gather, minus the `elem_size_bytes % 256 == 0`
    assert: that restriction belongs to the transpose path (256B xbar
    descriptors); the non-transpose Q7 ucode pushes descriptors with
    arbitrary byte lengths. Row *stride* must still be 256B-granular."""
    assert idxs_ap.dtype == mybir.dt.int16
    assert in_ap.space == MemorySpace.DRAM
    assert idxs_ap.space == MemorySpace.SBUF
    assert out_ap.space == MemorySpace.SBUF
    assert in_ap.dtype == out_ap.dtype
    assert ap_utils.ap_is_contiguous(out_ap.ap[1:])
    assert ap_utils.ap_is_contiguous(idxs_ap.ap[1:])
    assert in_ap.ap[-1][1] == out_ap.ap[-1][1] == elem_size
    assert out_ap.ap[0][1] * out_ap.ap[1][1] == num_idxs
    assert num_idxs % 128 == 0
    assert in_ap.ap[0][0] == elem_step
    stride_bytes = elem_step * mybir.dt.size(in_ap.dtype)
    stride_bytes_256 = stride_bytes // 256
    assert stride_bytes_256 * 256 == stride_bytes and stride_bytes_256 < 256
    _in_ap = gp.lower_ap_dma(in_ap, for_custom_bir_dma=True)
    _idxs_ap = gp.lower_ap(idxs_ap)
    _out_ap = gp.lower_ap(out_ap)
    return gp.add_instruction(
        mybir.InstDMAGatherAnt(
            name=gp.bass.get_next_instruction_name(),
            ins=[*_in_ap, _idxs_ap, gp.lower_val_access(gp.to_reg(num_idxs))],
            outs=[_out_ap],
            transpose=False,
            num_idxs=num_idxs,
            elem_size=elem_size,
            stride_bytes_256=stride_bytes_256,
            gen_mode=0,
            # single_packet concatenates each engine's whole descriptor
            # stream into one SDMA packet; the packet ceiling is 64
            # descriptors, so any gather over 1024 indices must use
            # per-descriptor packets.
            single_packet=single_packet,
            queue_num=queue_num,
            sbuf_tokens_per_rank=0,
            sbuf_free_dim_per_rank=0,
            sbuf_free_dim_pad_per_rank=0,
            sbuf_byte_offset=0,
        )
    )


def _build_table(nc, tabp, rot_d, tr_d, tab_w):
    """Compute tab[j] = [R(j) row-major (9) | t(j) (3)] into the first 12
    floats of each 64-float table row, for all 1024 slots (entries >= 1000
    are zero-padded -> R=I, never gathered)."""
    v = nc.vector

    rot_t = tabp.tile([P, 8, 3], f32)
    tr_t = tabp.tile([P, 8, 3], f32)
    v.memset(rot_t[:], 0.0)
    v.memset(tr_t[:], 0.0)
    # 1000 rows = 125 partitions x 8 entries
    nc.sync.dma_start(out=rot_t[0:125], in_=rot_d[:])
    nc.sync.dma_start(out=tr_t[0:125], in_=tr_d[:])

    tab_t = tabp.tile([P, 8, E], f32)

    def s(name):
        return tabp.tile([P, 8], f32, name=name, tag=name)

    rx = rot_t[:, :, 0]
    ry = rot_t[:, :, 1]
    rz = rot_t[:, :, 2]
    rx2, ry2, rz2 = s("rx2"), s("ry2"), s("rz2")
    th2, th4, th6 = s("th2"), s("th4"), s("th6")
    sc, cc, diag, tmp = s("sc"), s("cc"), s("diag"), s("tmp")
    pq, mq = s("pq"), s("mq")

    v.tensor_mul(rx2[:], rx, rx)
    v.tensor_mul(ry2[:], ry, ry)
    v.tensor_mul(rz2[:], rz, rz)
    v.tensor_add(th2[:], rx2[:], ry2[:])
    v.tensor_add(th2[:], th2[:], rz2[:])
    v.tensor_mul(th4[:], th2[:], th2[:])
    v.tensor_mul(th6[:], th4[:], th2[:])
    # sin_c = sin(th)/th       = 1 - th2/6 + th4/120 - th6/5040
    v.tensor_scalar(sc[:], th2[:], -1.0 / 6.0, 1.0, mult, add)
    v.tensor_scalar(tmp[:], th4[:], 1.0 / 120.0, None, mult)
    v.tensor_add(sc[:], sc[:], tmp[:])
    v.tensor_scalar(tmp[:], th6[:], -1.0 / 5040.0, None, mult)
    v.tensor_add(sc[:], sc[:], tmp[:])
    # cos_c = (1-cos(th))/th2  = 1/2 - th2/24 + th4/720 - th6/40320
    v.tensor_scalar(cc[:], th2[:], -1.0 / 24.0, 0.5, mult, add)
    v.tensor_scalar(tmp[:], th4[:], 1.0 / 720.0, None, mult)
    v.tensor_add(cc[:], cc[:], tmp[:])
    v.tensor_scalar(tmp[:], th6[:], -1.0 / 40320.0, None, mult)
    v.tensor_add(cc[:], cc[:], tmp[:])
    # R = (1 - cos_c*th2) I + sin_c K + cos_c r r^T
    v.tensor_mul(tmp[:], cc[:], th2[:])
    v.tensor_scalar(diag[:], tmp[:], -1.0, 1.0, mult, add)
    for a, r2 in ((0, rx2), (4, ry2), (8, rz2)):
        v.tensor_mul(tmp[:], cc[:], r2[:])
        v.tensor_add(tab_t[:, :, a], diag[:], tmp[:])
    for cm, cp, ra, rb, rk in (
        (1, 3, rx, ry, rz),   # R01 = cc rx ry - sc rz ; R10 = + sc rz
        (6, 2, rx, rz, ry),   # R20 = cc rx rz - sc ry ; R02 = + sc ry
        (5, 7, ry, rz, rx),   # R12 = cc ry rz - sc rx ; R21 = + sc rx
    ):
        v.tensor_mul(pq[:], ra, rb)
        v.tensor_mul(pq[:], cc[:], pq[:])
        v.tensor_mul(mq[:], sc[:], rk)
        v.tensor_tensor(tab_t[:, :, cm], pq[:], mq[:], subtract)
        v.tensor_tensor(tab_t[:, :, cp], pq[:], mq[:], add)
    for c in range(3):
        v.tensor_copy(tab_t[:, :, 9 + c], tr_t[:, :, c])

    nc.scalar.dma_start(out=tab_w, in_=tab_t[:])


def _build():
    nc = bacc.Bacc(
        "TRN2", target_bir_lowering=False, debug=False, num_swdge_queues=4
    )
    W = T // 16
    idx_d = nc.dram_tensor("idx", [NT, P, W], i16, kind="ExternalInput")
    o_d = nc.dram_tensor("o", [NT, P, J, 3], f32, kind="ExternalInput")
    d_d = nc.dram_tensor("d", [NT, P, J, 3], f32, kind="ExternalInput")
    rot_d = nc.dram_tensor("rot", [125, 8, 3], f32, kind="ExternalInput")
    tr_d = nc.dram_tensor("tr", [125, 8, 3], f32, kind="ExternalInput")
    no_d = nc.dram_tensor("no", [NT, P, J, 3], f32, kind="ExternalOutput")
    nd_d = nc.dram_tensor("nd", [NT, P, J, 3], f32, kind="ExternalOutput")
    R_d = nc.dram_tensor("R", [NT, P, J, 9], f32, kind="ExternalOutput")
    t_d = nc.dram_tensor("t", [NT, P, J, 3], f32, kind="ExternalOutput")

    with TileContext(nc) as tc:
        with (
            tc.tile_pool(name="dram", bufs=1, space="DRAM") as dpool,
            tc.tile_pool(name="tabp", bufs=1) as tabp,
            tc.tile_pool(name="work", bufs=2) as work,
        ):
            tab_dram = dpool.tile([NPAD, ROW], f32)
            tab_w = tab_dram[:].rearrange("(p c) e -> p c e", p=P)[:, :, 0:E]
            _build_table(nc, tabp, rot_d, tr_d, tab_w)
            tab_gather = tab_dram[:, 0:E]  # [1024, 12] rows, stride 64

            v = nc.vector
            for k in range(NT):
                idx_t = work.tile([P, W], i16, tag="idx")
                G = work.tile([P, J, E], f32, tag="G")
                o_t = work.tile([P, J, 3], f32, tag="o")
                dd_t = work.tile([P, J, 3], f32, tag="d")
                no_t = work.tile([P, J, 3], f32, tag="no")
                nd_t = work.tile([P, J, 3], f32, tag="nd")
                acc = work.tile([P, J], f32, tag="acc")
                tmp = work.tile([P, J], f32, tag="tmp")

                nc.sync.dma_start(out=idx_t[:], in_=idx_d[k])
                # chunk the gather: one call's descriptors must fit the
                # SWDGE ring (dynamic_dma_scratch_size/16 = 1024 descs per
                # engine); 8192 idxs = 513 descs leaves room to overlap.
                # Round-robin the 4 SWDGE queues - each queue is a separate
                # Q7 core pair, so descriptor generation (the measured
                # bottleneck, ~8.5ns/row/queue) runs 4-wide.
                CH = min(8192, T)
                for c in range(T // CH):
                    _dma_gather_raw(
                        nc.gpsimd,
                        out_ap=G[:, c * (CH // P):(c + 1) * (CH // P), :],
                        in_ap=tab_gather,
                        idxs_ap=idx_t[:, c * (CH // 16):(c + 1) * (CH // 16)],
                        num_idxs=CH,
                        elem_size=E,
                        elem_step=ROW,
                        queue_num=c % 4,
                    )
                nc.sync.dma_start(out=o_t[:], in_=o_d[k])
                nc.sync.dma_start(out=dd_t[:], in_=d_d[k])

                for a in range(3):
                    # new_o_a = sum_c R[a,c]*o_c + t_a
                    v.tensor_mul(acc[:], G[:, :, 3 * a], o_t[:, :, 0])
                    v.tensor_mul(tmp[:], G[:, :, 3 * a + 1], o_t[:, :, 1])
                    v.tensor_add(acc[:], acc[:], tmp[:])
                    v.tensor_mul(tmp[:], G[:, :, 3 * a + 2], o_t[:, :, 2])
                    v.tensor_add(acc[:], acc[:], tmp[:])
                    v.tensor_add(no_t[:, :, a], acc[:], G[:, :, 9 + a])
                    # new_d_a = sum_c R[a,c]*d_c
                    v.tensor_mul(acc[:], G[:, :, 3 * a], dd_t[:, :, 0])
                    v.tensor_mul(tmp[:], G[:, :, 3 * a + 1], dd_t[:, :, 1])
                    v.tensor_add(acc[:], acc[:], tmp[:])
                    v.tensor_mul(tmp[:], G[:, :, 3 * a + 2], dd_t[:, :, 2])
                    v.tensor_add(nd_t[:, :, a], acc[:], tmp[:])

                # compact the gathered R/t slices into contiguous tiles
                # before storing: a DMA from the strided G view generates a
                # descriptor per 36B/12B run (1M tiny HWDGE packets across
                # the kernel); two DVE copies make every store contiguous.
                Rc = work.tile([P, J, 9], f32, tag="Rc")
                tc_t = work.tile([P, J, 3], f32, tag="tc")
                v.tensor_copy(Rc[:], G[:, :, 0:9])
                v.tensor_copy(tc_t[:], G[:, :, 9:12])

                nc.scalar.dma_start(out=no_d[k], in_=no_t[:])
                nc.scalar.dma_start(out=nd_d[k], in_=nd_t[:])
                nc.scalar.dma_start(out=R_d[k], in_=Rc[:])
                nc.scalar.dma_start(out=t_d[k], in_=tc_t[:])

    nc.compile()
    return nc


def _prep_indices(i_core):
    """[BC] int -> [NT, 128, T/16] int16 in dma_gather's consumption order.

    Gather slot n lands at SBUF (partition n%128, chunk n//128); we want
    (p, j) to hold ray p*J + j, so slot n must carry ray (n%128)*J + n//128.
    The ucode then reads slot n from idx word (partition n%16, word n//16),
    replicated across the eight 16-partition groups.
    """
    ii = i_core.reshape(NT, P, J)
    perm = ii.transpose(0, 2, 1).reshape(NT, T)            # slot order
    w16 = perm.reshape(NT, T // 16, 16).transpose(0, 2, 1)  # [NT, 16, T/16]
    return np.tile(w16, (1, 8, 1)).astype(np.int16)         # [NT, 128, T/16]


def get_nc():
    if "nc" not in _CACHE:
        _CACHE["nc"] = _build()
    return _CACHE["nc"]


def kernel(i, o, d, rotation, translation, **run_kwargs):
    i = np.ascontiguousarray(np.asarray(i).astype(np.int64))
    o = np.ascontiguousarray(np.asarray(o, dtype=np.float32))
    d = np.ascontiguousarray(np.asarray(d, dtype=np.float32))
    rot = np.ascontiguousarray(np.asarray(rotation, dtype=np.float32)).reshape(125, 8, 3)
    tr = np.ascontiguousarray(np.asarray(translation, dtype=np.float32)).reshape(125, 8, 3)

    nc = get_nc()
    in_maps = []
    for c in range(N_CORES):
        sl = slice(c * BC, (c + 1) * BC)
        in_maps.append({
            "idx": _prep_indices(i[sl]),
            "o": o[sl].reshape(NT, P, J, 3),
            "d": d[sl].reshape(NT, P, J, 3),
            "rot": rot,
            "tr": tr,
        })
    res = run_bass_kernel_spmd(nc, in_maps, core_ids=list(range(N_CORES)), **run_kwargs)
    _CACHE["last_result"] = res

    new_o = np.concatenate([res.results[c]["no"].reshape(BC, 3) for c in range(N_CORES)])
    new_d = np.concatenate([res.results[c]["nd"].reshape(BC, 3) for c in range(N_CORES)])
    R = np.concatenate([res.results[c]["R"].reshape(BC, 3, 3) for c in range(N_CORES)])
    t = np.concatenate([res.results[c]["t"].reshape(BC, 3) for c in range(N_CORES)])
    return new_o, new_d, R, t
